# revision 31
# baseline (speedup 1.0000x reference)
"""Trainium2 Bass kernel for a 2-layer hetero GCN (nn_NetGCN).

Math (per relation r with edges (src, dst), weights W, bias b):
    y = relu?( Dk^-1/2 * segsum_dst( (Do^-1/2 * x)[src] ) @ W + b )
Layer 1: y_i + y_b (relations 'interacts' and 'behave', relu inside each).
Layer 2: relation 'interacts' on h, no relu.

Distribution: edges sharded by dst across 8 cores (each core owns a
contiguous 12544-node slice).  Each core gathers source rows (bf16) from a
replicated node-feature table with `dma_gather`, segment-sums them into PSUM
via one-hot matmuls on TensorE, applies norm/weights, and the h-table is
AllGathered between the layers.

Slots are laid out in supergroups of SG=7 dst-blocks so each (supergroup,
chunk) is ONE big dma_gather call (168/stage instead of 1176) and each
supergroup is ONE is_equal one-hot build on DVE; block boundaries inside a
gather region are handled with partition-range matmul fragments accumulating
into per-block PSUM tiles.
"""

import sys

sys.path.insert(0, "/opt/trn_rl_repo")

import numpy as np

P = 128
NCORES = 8
NCHUNK = 4
SG = 7  # dst blocks per supergroup

_PROGRAM_CACHE = {}


def _host_prep_relation(src, dst, npad, shard, nblk, chunk, chunk_map=None):
    """Sort one relation's edges by (dst-block, src-chunk); build per-core
    int16 gather indices and per-slot dst-local columns in supergroup layout.

    Slot order: for sg: for chunk k: for block b in sg: R[b,k] slots
    (R = max edge count over cores), each (sg,k) region padded to 128.

    Returns (idx16 [NCORES,128,S//16], dstloc [NCORES,128,S//128],
             layout dict with R, rows_sgk, reg_len, reg_off, off_bk)
    """
    blk = dst // P
    if chunk_map is None:
        chk = src // chunk
        loc = src - chk * chunk
    else:
        chk, loc = chunk_map
    order = np.lexsort((chk, blk))
    loc_s = loc[order]
    dst_s = dst[order]

    nblk_tot = npad // P
    grp = blk[order] * NCHUNK + chk[order]
    counts = np.bincount(grp, minlength=nblk_tot * NCHUNK).reshape(
        nblk_tot, NCHUNK
    )
    bpc = nblk
    counts_c = counts.reshape(NCORES, bpc, NCHUNK)
    R = counts_c.max(axis=0).astype(np.int64)          # [bpc, NCHUNK]
    # 128-align block regions: every matmul fragment is a full base-0 tile
    R = -(-R // 128) * 128

    nsg = bpc // SG
    assert nsg * SG == bpc
    rows_sgk = R.reshape(nsg, SG, NCHUNK).sum(axis=1)  # [nsg, NCHUNK]
    reg_len = -(-rows_sgk // P) * P                    # [nsg, NCHUNK]

    S = int(reg_len.sum())
    reg_off = np.zeros((nsg, NCHUNK), dtype=np.int64)
    pos = 0
    for s_ in range(nsg):
        for k in range(NCHUNK):
            reg_off[s_, k] = pos
            pos += int(reg_len[s_, k])
    off_bk = np.zeros((bpc, NCHUNK), dtype=np.int64)
    for s_ in range(nsg):
        for k in range(NCHUNK):
            cur = int(reg_off[s_, k])
            for j in range(SG):
                b = s_ * SG + j
                off_bk[b, k] = cur
                cur += int(R[b, k])

    grp_start = np.zeros(nblk_tot * NCHUNK + 1, dtype=np.int64)
    np.cumsum(counts.ravel(), out=grp_start[1:])
    counts_r = counts.ravel()

    idx16 = np.zeros((NCORES, S), dtype=np.int16)
    dstloc = np.full((NCORES, S), -1.0, dtype=np.float32)
    for c in range(NCORES):
        for b in range(bpc):
            gb = c * bpc + b
            for k in range(NCHUNK):
                g = gb * NCHUNK + k
                n = int(counts_r[g])
                if n:
                    e0 = int(grp_start[g])
                    s0 = int(off_bk[b, k])
                    idx16[c, s0 : s0 + n] = loc_s[e0 : e0 + n].astype(np.int16)
                    dstloc[c, s0 : s0 + n] = (
                        dst_s[e0 : e0 + n] - (c * shard + b * P)
                    ).astype(np.float32)

    # wrap: idx j -> [j % 16, j // 16], replicated to all 8 Q7 core groups
    idx_w = np.ascontiguousarray(
        np.tile(idx16.reshape(NCORES, S // 16, 16).transpose(0, 2, 1), (1, 8, 1))
    )
    # dstloc: slot j -> [j % 128, j // 128]
    dst_w = np.ascontiguousarray(
        dstloc.reshape(NCORES, S // P, P).transpose(0, 2, 1)
    )
    layout = dict(R=R, rows_sgk=rows_sgk, reg_len=reg_len,
                  reg_off=reg_off, off_bk=off_bk)
    return idx_w, dst_w, layout


def _build_program(npad, shard, nblk, chunk, s_i, s_b, lay_i, lay_b, has_bias):
    import concourse.bacc as bacc
    import concourse.tile as tile
    from concourse import library_config, mybir

    f32 = mybir.dt.float32
    bf16 = mybir.dt.bfloat16
    i16 = mybir.dt.int16
    AF = mybir.ActivationFunctionType
    ALU = mybir.AluOpType

    nsg = nblk // SG

    nc = bacc.Bacc(
        "TRN2",
        target_bir_lowering=False,
        debug=False,
        num_devices=NCORES,
        num_swdge_queues=4,
    )

    xn_i = nc.declare_dram_parameter("xn_i", [npad, P], bf16, isOutput=False)
    xn_b = nc.declare_dram_parameter("xn_b", [npad, P], bf16, isOutput=False)
    idx_i = nc.declare_dram_parameter("idx_i", [P, s_i // 16], i16, isOutput=False)
    idx_b = nc.declare_dram_parameter("idx_b", [P, s_b // 16], i16, isOutput=False)
    dl_i = nc.declare_dram_parameter("dl_i", [P, s_i // P], bf16, isOutput=False)
    dl_b = nc.declare_dram_parameter("dl_b", [P, s_b // P], bf16, isOutput=False)
    rin_i = nc.declare_dram_parameter("rin_i", [P, nblk], f32, isOutput=False)
    rin_b = nc.declare_dram_parameter("rin_b", [P, nblk], f32, isOutput=False)
    rout2 = nc.declare_dram_parameter("rout2", [P, nblk], f32, isOutput=False)
    rvrows = 65
    rvcols = -(-nblk // 3) * P
    rvi = nc.declare_dram_parameter("rvi", [rvrows, rvcols], f32, isOutput=False)
    rvb = nc.declare_dram_parameter("rvb", [rvrows, rvcols], f32, isOutput=False)
    w1i = nc.declare_dram_parameter("w1i", [P, P], bf16, isOutput=False)
    w1b = nc.declare_dram_parameter("w1b", [P, P], bf16, isOutput=False)
    w2 = nc.declare_dram_parameter("w2", [P, P], bf16, isOutput=False)
    b1i = nc.declare_dram_parameter("b1i", [rvrows, P], f32, isOutput=False)
    b1b = nc.declare_dram_parameter("b1b", [rvrows, P], f32, isOutput=False)
    b2 = nc.declare_dram_parameter("b2", [rvrows, P], f32, isOutput=False)
    iota_in = nc.declare_dram_parameter("iota", [P, P], bf16, isOutput=False)
    y_out = nc.declare_dram_parameter("y", [shard, P], f32, isOutput=True)

    hn_shard = nc.dram_tensor("hn_shard", [shard, P], bf16)
    hn_full = nc.dram_tensor("hn_full", [npad, P], bf16, addr_space="Shared")

    max_sg_len = 0
    max_icols = 0
    for lay in (lay_i, lay_b):
        for s_ in range(nsg):
            max_sg_len = max(max_sg_len, int(lay["reg_len"][s_].sum()))
            for k in range(NCHUNK):
                max_icols = max(
                    max_icols, -(-int(lay["rows_sgk"][s_, k]) // 16)
                )

    with tile.TileContext(nc) as tc:
        nc.gpsimd.load_library(library_config.mlp)
        with (
            tc.tile_pool(name="cst", bufs=1) as cst,
            tc.tile_pool(name="edg", bufs=10) as edg,
            tc.tile_pool(name="gp", bufs=2) as gp,
            tc.tile_pool(name="sp", bufs=2) as sp,
            tc.tile_pool(name="bp", bufs=4) as bp,
            tc.tile_pool(name="pa", bufs=4, space="PSUM") as pa,
            tc.tile_pool(name="py", bufs=2, space="PSUM") as py,
        ):
            def load_cst(t, shape, dtype=f32):
                s = cst.tile(list(shape), dtype, tag=t.name)
                nc.sync.dma_start(out=s[:], in_=t[:])
                return s

            iota_sb = load_cst(iota_in, [P, P], bf16)
            w1i_sb = load_cst(w1i, [P, P], bf16)
            w1b_sb = load_cst(w1b, [P, P], bf16)
            w2_sb = load_cst(w2, [P, P], bf16)
            b1i_sb = load_cst(b1i, [rvrows, P])
            b1b_sb = load_cst(b1b, [rvrows, P])
            b2_sb = load_cst(b2, [rvrows, P])
            rini_sb = load_cst(rin_i, [P, nblk])
            rinb_sb = load_cst(rin_b, [P, nblk])
            rout_sb = load_cst(rout2, [P, nblk])
            rvi_sb = load_cst(rvi, [rvrows, rvcols]) if has_bias else None
            rvb_sb = load_cst(rvb, [rvrows, rvcols]) if has_bias else None
            dli_sb = load_cst(dl_i, [P, s_i // P], bf16)
            dlb_sb = load_cst(dl_b, [P, s_b // P], bf16)

            h_buf = cst.tile([P, nblk * P], bf16, tag="h_buf")
            qctr = [0]

            def stage(tables, idx_t, dl_sb, lay, w_sb, bias_sb, rinv_sb,
                      epilogue):
                R = lay["R"]
                rows_sgk = lay["rows_sgk"]
                reg_len = lay["reg_len"]
                reg_off = lay["reg_off"]
                off_bk = lay["off_bk"]
                for s_ in range(nsg):
                    sg_off = int(reg_off[s_, 0])
                    sg_len = int(reg_len[s_].sum())
                    tksg = sg_len // P

                    g_sg = gp.tile(
                        [P, sg_len], bf16, tag="g",
                        padded_shape=[P, max_sg_len],
                    )
                    SUB = 1024  # single-packet limit: 64 desc x 16 engines
                    for k in range(NCHUNK):
                        rows = int(rows_sgk[s_, k])
                        if rows == 0:
                            continue
                        col0 = int(reg_off[s_, k]) - sg_off
                        for q in range(0, rows, SUB):
                            rq = min(SUB, rows - q)
                            rl = -(-rq // P) * P
                            icols = -(-rq // 16)
                            idx_sb = edg.tile(
                                [P, icols], i16, tag="idx",
                                padded_shape=[P, SUB // 16],
                            )
                            nc.sync.dma_start(
                                out=idx_sb[:],
                                in_=idx_t[
                                    :,
                                    (sg_off + col0 + q) // 16 :
                                    (sg_off + col0 + q) // 16 + icols,
                                ],
                            )
                            nc.gpsimd.dma_gather(
                                out_ap=g_sg[
                                    :, col0 + q : col0 + q + rl
                                ].rearrange("p (t d) -> p t d", d=P),
                                in_ap=tables[k],
                                idxs_ap=idx_sb[:],
                                num_idxs=rq,
                                num_idxs_reg=rq,
                                elem_size=P,
                                queue_num=qctr[0] % 4,
                            )
                            qctr[0] += 1

                    s_t = sp.tile(
                        [P, sg_len], bf16, tag="s",
                        padded_shape=[P, max_sg_len],
                    )
                    nc.vector.tensor_tensor(
                        out=s_t[:].rearrange("p (t n) -> p t n", n=P),
                        in0=dl_sb[:, sg_off // P : sg_off // P + tksg]
                        .unsqueeze(2)
                        .to_broadcast([P, tksg, P]),
                        in1=iota_sb[:].unsqueeze(1).to_broadcast([P, tksg, P]),
                        op=ALU.is_equal,
                    )

                    # fragment schedule: per block, list of (col, c0, c1);
                    # 64-aligned regions -> fragment base is always 0 or 64
                    frags = [[] for _ in range(SG)]
                    for k in range(NCHUNK):
                        col0 = int(reg_off[s_, k]) - sg_off
                        for j in range(SG):
                            b = s_ * SG + j
                            r = int(R[b, k])
                            if r == 0:
                                continue
                            s0 = int(off_bk[b, k]) - int(reg_off[s_, k])
                            s1 = s0 + r
                            for t in range(s0 // P, -(-s1 // P)):
                                c0 = max(s0 - t * P, 0)
                                c1 = min(s1 - t * P, P)
                                frags[j].append((col0 + t * P, c0, c1))

                    # one PSUM bank per block; sequential accumulation groups
                    # (only one open matmul accumulation group per bank)
                    aggs = []
                    for j in range(SG):
                        a = pa.tile([P, P], f32, tag="agg")
                        aggs.append(a)
                        if not frags[j]:
                            nc.vector.memset(a[:], 0.0)
                        for fi, (col, c0, c1) in enumerate(frags[j]):
                            nc.tensor.matmul(
                                out=a[:],
                                lhsT=g_sg[c0:c1, col : col + P],
                                rhs=s_t[c0:c1, col : col + P],
                                start=(fi == 0),
                                stop=(fi == len(frags[j]) - 1),
                            )

                    for j in range(SG):
                        b = s_ * SG + j
                        aggT = bp.tile([P, P], bf16, tag="aggT")
                        nc.scalar.copy(out=aggT[:], in_=aggs[j][:])
                        y_ps = py.tile([P, P], f32, tag="yps")
                        nc.tensor.matmul(
                            out=y_ps[:], lhsT=aggT[:], rhs=w_sb[:],
                            start=True, stop=not has_bias,
                        )
                        if has_bias:
                            nc.tensor.matmul(
                                out=y_ps[:],
                                lhsT=rinv_sb[
                                    (b % 3) * 32 : (b % 3) * 32 + 1,
                                    (b // 3) * P : (b // 3) * P + P,
                                ],
                                rhs=bias_sb[(b % 3) * 32 : (b % 3) * 32 + 1, :],
                                start=False, stop=True,
                            )
                        epilogue(b, y_ps)

            def epi_l1i(b, y_ps):
                nc.scalar.activation(
                    out=h_buf[:, b * P : (b + 1) * P], in_=y_ps[:], func=AF.Relu,
                    scale=rini_sb[:, b : b + 1],
                )

            def epi_l1b(b, y_ps):
                rb = bp.tile([P, P], bf16, tag="rb")
                nc.scalar.activation(
                    out=rb[:], in_=y_ps[:], func=AF.Relu,
                    scale=rinb_sb[:, b : b + 1],
                )
                hs = h_buf[:, b * P : (b + 1) * P]
                nc.vector.tensor_tensor(out=hs, in0=hs, in1=rb[:], op=ALU.add)
                hn = bp.tile([P, P], bf16, tag="hn")
                nc.scalar.activation(
                    out=hn[:], in_=hs, func=AF.Copy, scale=rout_sb[:, b : b + 1]
                )
                nc.sync.dma_start(out=hn_shard[b * P : (b + 1) * P, :], in_=hn[:])

            def epi_l2(b, y_ps):
                ob = bp.tile([P, P], f32, tag="ob")
                nc.scalar.activation(
                    out=ob[:], in_=y_ps[:], func=AF.Copy,
                    scale=rini_sb[:, b : b + 1],
                )
                nc.sync.dma_start(out=y_out[b * P : (b + 1) * P, :], in_=ob[:])

            xt = [xn_i[k * chunk : (k + 1) * chunk, :] for k in range(NCHUNK)]
            xtb = [xn_b[k * chunk : (k + 1) * chunk, :] for k in range(NCHUNK)]
            ht = [hn_full[k * chunk : (k + 1) * chunk, :] for k in range(NCHUNK)]
            stage(xt, idx_i, dli_sb, lay_i, w1i_sb, b1i_sb, rvi_sb, epi_l1i)
            stage(xtb, idx_b, dlb_sb, lay_b, w1b_sb, b1b_sb, rvb_sb, epi_l1b)
            nc.gpsimd.collective_compute(
                "AllGather",
                mybir.AluOpType.bypass,
                replica_groups=[list(range(NCORES))],
                ins=[hn_shard[:]],
                outs=[hn_full[:]],
            )
            stage(ht, idx_i, dli_sb, lay_i, w2_sb, b2_sb, rvi_sb, epi_l2)

    nc.compile()
    return nc


def kernel(x, src_i, dst_i, src_b, dst_b, W1_i, b1_i, W1_b, b1_b, W2, b2):
    import ml_dtypes

    from concourse.bass_utils import run_bass_kernel_spmd

    bf16 = ml_dtypes.bfloat16
    x = np.asarray(x, dtype=np.float32)
    src_i = np.asarray(src_i, dtype=np.int64)
    dst_i = np.asarray(dst_i, dtype=np.int64)
    src_b = np.asarray(src_b, dtype=np.int64)
    dst_b = np.asarray(dst_b, dtype=np.int64)
    W1_i = np.asarray(W1_i, dtype=np.float32)
    b1_i = np.asarray(b1_i, dtype=np.float32)
    W1_b = np.asarray(W1_b, dtype=np.float32)
    b1_b = np.asarray(b1_b, dtype=np.float32)
    W2 = np.asarray(W2, dtype=np.float32)
    b2 = np.asarray(b2, dtype=np.float32)

    n = x.shape[0]
    npad = -(-n // (NCORES * P)) * (NCORES * P)
    shard = npad // NCORES
    nblk = shard // P
    chunk = npad // NCHUNK
    assert chunk <= 32768 and chunk % 16 == 0

    def degs(idx):
        d = np.bincount(idx, minlength=npad).astype(np.float32)
        return np.maximum(d, 1.0) ** -0.5

    ro_i = degs(src_i)
    ri_i = degs(dst_i)
    ro_b = degs(src_b)
    ri_b = degs(dst_b)

    xn_i = np.zeros((npad, P), dtype=bf16)
    xn_i[:n] = (x * ro_i[:n, None]).astype(bf16)
    xn_b = np.zeros((npad, P), dtype=bf16)
    xn_b[:n] = (x * ro_b[:n, None]).astype(bf16)

    idx_i, dl_i, lay_i = _host_prep_relation(
        src_i, dst_i, npad, shard, nblk, chunk
    )
    idx_b, dl_b, lay_b = _host_prep_relation(
        src_b, dst_b, npad, shard, nblk, chunk
    )
    s_i = idx_i.shape[2] * 16
    s_b = idx_b.shape[2] * 16

    rin_i = ri_i.reshape(NCORES, nblk, P).transpose(0, 2, 1).copy()
    rin_b = ri_b.reshape(NCORES, nblk, P).transpose(0, 2, 1).copy()
    rout2 = ro_i.reshape(NCORES, nblk, P).transpose(0, 2, 1).copy()

    def pack_rv(r):
        # block b -> partition (b % 3) * 32, cols (b // 3)*128 .. +128
        ngrp = -(-nblk // 3)
        out = np.zeros((NCORES, 65, ngrp * P), dtype=np.float32)
        rb = (1.0 / r).astype(np.float32).reshape(NCORES, nblk, P)
        for b in range(nblk):
            out[:, (b % 3) * 32, (b // 3) * P : (b // 3) * P + P] = rb[:, b, :]
        return out

    rvi_h = pack_rv(ri_i)
    rvb_h = pack_rv(ri_b)

    has_bias = bool(np.any(b1_i) or np.any(b1_b) or np.any(b2))
    key = (npad, s_i, s_b, has_bias,
           lay_i["R"].tobytes(), lay_b["R"].tobytes())
    if key not in _PROGRAM_CACHE:
        _PROGRAM_CACHE.clear()
        _PROGRAM_CACHE[key] = _build_program(
            npad, shard, nblk, chunk, s_i, s_b, lay_i, lay_b, has_bias
        )
    nc = _PROGRAM_CACHE[key]

    def bias_rep(b):
        out = np.zeros((65, P), dtype=np.float32)
        out[0] = out[32] = out[64] = b
        return out

    iota = np.tile(np.arange(P, dtype=np.float32), (P, 1)).astype(bf16)

    dl_i = dl_i.astype(bf16)
    dl_b = dl_b.astype(bf16)
    W1_i_h = W1_i.astype(bf16)
    W1_b_h = W1_b.astype(bf16)
    W2_h = W2.astype(bf16)

    in_maps = []
    for c in range(NCORES):
        in_maps.append(
            {
                "xn_i": xn_i,
                "xn_b": xn_b,
                "idx_i": idx_i[c],
                "idx_b": idx_b[c],
                "dl_i": dl_i[c],
                "dl_b": dl_b[c],
                "rin_i": rin_i[c],
                "rin_b": rin_b[c],
                "rout2": rout2[c],
                "rvi": rvi_h[c],
                "rvb": rvb_h[c],
                "w1i": W1_i_h,
                "w1b": W1_b_h,
                "w2": W2_h,
                "b1i": bias_rep(b1_i),
                "b1b": bias_rep(b1_b),
                "b2": bias_rep(b2),
                "iota": iota,
            }
        )

    import os

    trace = os.environ.get("GCN_TRACE", "0") == "1"
    res = run_bass_kernel_spmd(
        nc, in_maps, core_ids=list(range(NCORES)), trace=trace
    )
    if trace and res.exec_time_ns:
        print(f"HW exec time: {res.exec_time_ns} ns")
    y = np.concatenate([res.results[c]["y"] for c in range(NCORES)], axis=0)
    return y[:n]


# revision 36
# speedup vs baseline: 1.2523x; 1.2523x over previous
"""Trainium2 Bass kernel for a 2-layer hetero GCN (nn_NetGCN).

Math (per relation r with edges (src, dst), weights W, bias b):
    y = relu?( Dk^-1/2 * segsum_dst( (Do^-1/2 * x)[src] ) @ W + b )
Layer 1: y_i + y_b (relations 'interacts' and 'behave', relu inside each).
Layer 2: relation 'interacts' on h, no relu.

Distribution: edges sharded by dst across 8 cores (each core owns a
contiguous 12544-node slice).  Each core gathers source rows (bf16) from a
replicated node-feature table with `dma_gather`, segment-sums them into PSUM
via one-hot matmuls on TensorE, applies norm/weights, and the h-table is
AllGathered between the layers.

Slots are laid out in supergroups of SG=7 dst-blocks so each (supergroup,
chunk) is ONE big dma_gather call (168/stage instead of 1176) and each
supergroup is ONE is_equal one-hot build on DVE; block boundaries inside a
gather region are handled with partition-range matmul fragments accumulating
into per-block PSUM tiles.
"""

import sys

sys.path.insert(0, "/opt/trn_rl_repo")

import numpy as np

P = 128
NCORES = 8
NCHUNK = 4
SG = 7  # dst blocks per supergroup

_PROGRAM_CACHE = {}


def _host_prep_relation(src, dst, npad, shard, nblk, chunk, chunk_map=None):
    """Sort one relation's edges by (dst-block, src-chunk); build per-core
    int16 gather indices and per-slot dst-local columns in supergroup layout.

    Slot order: for sg: for chunk k: for block b in sg: R[b,k] slots
    (R = max edge count over cores), each (sg,k) region padded to 128.

    Returns (idx16 [NCORES,128,S//16], dstloc [NCORES,128,S//128],
             layout dict with R, rows_sgk, reg_len, reg_off, off_bk)
    """
    blk = dst // P
    if chunk_map is None:
        chk = src // chunk
        loc = src - chk * chunk
    else:
        chk, loc = chunk_map
    order = np.lexsort((chk, blk))
    loc_s = loc[order]
    dst_s = dst[order]

    nblk_tot = npad // P
    grp = blk[order] * NCHUNK + chk[order]
    counts = np.bincount(grp, minlength=nblk_tot * NCHUNK).reshape(
        nblk_tot, NCHUNK
    )
    bpc = nblk
    counts_c = counts.reshape(NCORES, bpc, NCHUNK)
    R = counts_c.max(axis=0).astype(np.int64)          # [bpc, NCHUNK]
    # 128-align block regions: every matmul fragment is a full base-0 tile
    R = -(-R // 128) * 128

    nsg = bpc // SG
    assert nsg * SG == bpc
    rows_sgk = R.reshape(nsg, SG, NCHUNK).sum(axis=1)  # [nsg, NCHUNK]
    reg_len = -(-rows_sgk // P) * P                    # [nsg, NCHUNK]

    S = int(reg_len.sum())
    reg_off = np.zeros((nsg, NCHUNK), dtype=np.int64)
    pos = 0
    for s_ in range(nsg):
        for k in range(NCHUNK):
            reg_off[s_, k] = pos
            pos += int(reg_len[s_, k])
    off_bk = np.zeros((bpc, NCHUNK), dtype=np.int64)
    for s_ in range(nsg):
        for k in range(NCHUNK):
            cur = int(reg_off[s_, k])
            for j in range(SG):
                b = s_ * SG + j
                off_bk[b, k] = cur
                cur += int(R[b, k])

    grp_start = np.zeros(nblk_tot * NCHUNK + 1, dtype=np.int64)
    np.cumsum(counts.ravel(), out=grp_start[1:])
    counts_r = counts.ravel()

    idx16 = np.zeros((NCORES, S), dtype=np.int16)
    dstloc = np.full((NCORES, S), -1.0, dtype=np.float32)
    for c in range(NCORES):
        for b in range(bpc):
            gb = c * bpc + b
            for k in range(NCHUNK):
                g = gb * NCHUNK + k
                n = int(counts_r[g])
                if n:
                    e0 = int(grp_start[g])
                    s0 = int(off_bk[b, k])
                    idx16[c, s0 : s0 + n] = loc_s[e0 : e0 + n].astype(np.int16)
                    dstloc[c, s0 : s0 + n] = (
                        dst_s[e0 : e0 + n] - (c * shard + b * P)
                    ).astype(np.float32)

    # wrap: idx j -> [j % 16, j // 16], replicated to all 8 Q7 core groups
    idx_w = np.ascontiguousarray(
        np.tile(idx16.reshape(NCORES, S // 16, 16).transpose(0, 2, 1), (1, 8, 1))
    )
    # dstloc: slot j -> [j % 128, j // 128]
    dst_w = np.ascontiguousarray(
        dstloc.reshape(NCORES, S // P, P).transpose(0, 2, 1)
    )
    layout = dict(R=R, rows_sgk=rows_sgk, reg_len=reg_len,
                  reg_off=reg_off, off_bk=off_bk)
    return idx_w, dst_w, layout


def _build_program(npad, shard, nblk, chunk, s_i, s_b, lay_i, lay_b, has_bias):
    import concourse.bacc as bacc
    import concourse.tile as tile
    from concourse import library_config, mybir

    f32 = mybir.dt.float32
    bf16 = mybir.dt.bfloat16
    i16 = mybir.dt.int16
    AF = mybir.ActivationFunctionType
    ALU = mybir.AluOpType

    nsg = nblk // SG

    nc = bacc.Bacc(
        "TRN2",
        target_bir_lowering=False,
        debug=False,
        num_devices=NCORES,
        num_swdge_queues=4,
    )

    xn_i = nc.declare_dram_parameter("xn_i", [npad, P], bf16, isOutput=False)
    xn_b = nc.declare_dram_parameter("xn_b", [npad, P], bf16, isOutput=False)
    idx_i = nc.declare_dram_parameter("idx_i", [P, s_i // 16], i16, isOutput=False)
    idx_b = nc.declare_dram_parameter("idx_b", [P, s_b // 16], i16, isOutput=False)
    dl_i = nc.declare_dram_parameter("dl_i", [P, s_i // P], bf16, isOutput=False)
    dl_b = nc.declare_dram_parameter("dl_b", [P, s_b // P], bf16, isOutput=False)
    rin_i = nc.declare_dram_parameter("rin_i", [P, nblk], f32, isOutput=False)
    rin_b = nc.declare_dram_parameter("rin_b", [P, nblk], f32, isOutput=False)
    rout2 = nc.declare_dram_parameter("rout2", [P, nblk], f32, isOutput=False)
    rvrows = 65
    rvcols = -(-nblk // 3) * P
    rvi = nc.declare_dram_parameter("rvi", [rvrows, rvcols], f32, isOutput=False)
    rvb = nc.declare_dram_parameter("rvb", [rvrows, rvcols], f32, isOutput=False)
    w1i = nc.declare_dram_parameter("w1i", [P, P], bf16, isOutput=False)
    w1b = nc.declare_dram_parameter("w1b", [P, P], bf16, isOutput=False)
    w2 = nc.declare_dram_parameter("w2", [P, P], bf16, isOutput=False)
    b1i = nc.declare_dram_parameter("b1i", [rvrows, P], f32, isOutput=False)
    b1b = nc.declare_dram_parameter("b1b", [rvrows, P], f32, isOutput=False)
    b2 = nc.declare_dram_parameter("b2", [rvrows, P], f32, isOutput=False)
    iota_in = nc.declare_dram_parameter("iota", [P, P], bf16, isOutput=False)
    y_out = nc.declare_dram_parameter("y", [shard, P], f32, isOutput=True)

    hn_shard = nc.dram_tensor("hn_shard", [shard, P], bf16)
    hn_full = nc.dram_tensor("hn_full", [npad, P], bf16, addr_space="Shared")

    max_sg_len = 0
    max_icols = 0
    for lay in (lay_i, lay_b):
        for s_ in range(nsg):
            max_sg_len = max(max_sg_len, int(lay["reg_len"][s_].sum()))
            for k in range(NCHUNK):
                max_icols = max(
                    max_icols, -(-int(lay["rows_sgk"][s_, k]) // 16)
                )

    with tile.TileContext(nc) as tc:
        nc.gpsimd.load_library(library_config.mlp)
        with (
            tc.tile_pool(name="cst", bufs=1) as cst,
            tc.tile_pool(name="edg", bufs=2) as edg,
            tc.tile_pool(name="gp", bufs=6) as gp,
            tc.tile_pool(name="sp", bufs=4) as sp,
            tc.tile_pool(name="bp", bufs=4) as bp,
            tc.tile_pool(name="pa", bufs=SG, space="PSUM") as pa,
            tc.tile_pool(name="py", bufs=1, space="PSUM") as py,
        ):
            def load_cst(t, shape, dtype=f32):
                s = cst.tile(list(shape), dtype, tag=t.name)
                nc.sync.dma_start(out=s[:], in_=t[:])
                return s

            iota_sb = load_cst(iota_in, [P, P], bf16)
            w1i_sb = load_cst(w1i, [P, P], bf16)
            w1b_sb = load_cst(w1b, [P, P], bf16)
            w2_sb = load_cst(w2, [P, P], bf16)
            b1i_sb = load_cst(b1i, [rvrows, P])
            b1b_sb = load_cst(b1b, [rvrows, P])
            b2_sb = load_cst(b2, [rvrows, P])
            rini_sb = load_cst(rin_i, [P, nblk])
            rinb_sb = load_cst(rin_b, [P, nblk])
            rout_sb = load_cst(rout2, [P, nblk])
            rvi_sb = load_cst(rvi, [rvrows, rvcols]) if has_bias else None
            rvb_sb = load_cst(rvb, [rvrows, rvcols]) if has_bias else None
            dli_sb = load_cst(dl_i, [P, s_i // P], bf16)
            dlb_sb = load_cst(dl_b, [P, s_b // P], bf16)

            h_buf = cst.tile([P, nblk * P], bf16, tag="h_buf")
            qctr = [0]

            def stage(tables, idx_t, s_len, dl_sb, lay, w_sb, bias_sb, rinv_sb,
                      epilogue):
                R = lay["R"]
                rows_sgk = lay["rows_sgk"]
                reg_len = lay["reg_len"]
                reg_off = lay["reg_off"]
                off_bk = lay["off_bk"]

                # whole-stage index table resident in SBUF (one large load)
                idx_res = cst.tile(
                    [P, s_len // 16], i16, tag="idx_res", bufs=2,
                    padded_shape=[P, max(s_i, s_b) // 16],
                )
                nc.sync.dma_start(out=idx_res[:], in_=idx_t[:])

                SUB = 1024  # single-packet limit: 64 desc x 16 engines

                def transform_block(b, agg):
                    aggT = bp.tile([P, P], bf16, tag="aggT")
                    nc.scalar.copy(out=aggT[:], in_=agg[:])
                    y_ps = py.tile([P, P], f32, tag="yps")
                    nc.tensor.matmul(
                        out=y_ps[:], lhsT=aggT[:], rhs=w_sb[:],
                        start=True, stop=not has_bias,
                    )
                    if has_bias:
                        nc.tensor.matmul(
                            out=y_ps[:],
                            lhsT=rinv_sb[
                                (b % 3) * 32 : (b % 3) * 32 + 1,
                                (b // 3) * P : (b // 3) * P + P,
                            ],
                            rhs=bias_sb[(b % 3) * 32 : (b % 3) * 32 + 1, :],
                            start=False, stop=True,
                        )
                    epilogue(b, y_ps)

                for s_ in range(nsg):
                    # gathers: one tile per (sg, chunk) region, split <=1024
                    g_regs = []
                    for k in range(NCHUNK):
                        rows = int(rows_sgk[s_, k])
                        rl = int(reg_len[s_, k])
                        if rows == 0:
                            g_regs.append(None)
                            continue
                        off = int(reg_off[s_, k])
                        g_k = gp.tile(
                            [P, rl], bf16, tag="g",
                            padded_shape=[P, int(reg_len.max())],
                        )
                        g_regs.append(g_k)
                        for q in range(0, rows, SUB):
                            rq = min(SUB, rows - q)
                            nc.gpsimd.dma_gather(
                                out_ap=g_k[:, q : q + rq].rearrange(
                                    "p (t d) -> p t d", d=P
                                ),
                                in_ap=tables[k],
                                idxs_ap=idx_res[
                                    :, (off + q) // 16 : (off + q + rq) // 16
                                ],
                                num_idxs=rq,
                                num_idxs_reg=rq,
                                elem_size=P,
                                queue_num=qctr[0] % 4,
                            )
                            qctr[0] += 1

                    # one-hot per region on DVE
                    s_regs = []
                    for k in range(NCHUNK):
                        rl = int(reg_len[s_, k])
                        if rl == 0:
                            s_regs.append(None)
                            continue
                        off = int(reg_off[s_, k])
                        tkr = rl // P
                        s_k = sp.tile(
                            [P, rl], bf16, tag="s",
                            padded_shape=[P, int(reg_len.max())],
                        )
                        s_regs.append(s_k)
                        nc.vector.tensor_tensor(
                            out=s_k[:].rearrange("p (t n) -> p t n", n=P),
                            in0=dl_sb[:, off // P : off // P + tkr]
                            .unsqueeze(2)
                            .to_broadcast([P, tkr, P]),
                            in1=iota_sb[:]
                            .unsqueeze(1)
                            .to_broadcast([P, tkr, P]),
                            op=ALU.is_equal,
                        )

                    # region-major matmuls, one PSUM bank per block; a
                    # block's accumulation group stays open across regions
                    nmm = [
                        sum(int(R[s_ * SG + j, k]) // P for k in range(NCHUNK))
                        for j in range(SG)
                    ]
                    aggs = [
                        pa.tile([P, P], f32, tag="agg", name="agg")
                        for _ in range(SG)
                    ]
                    done = [0] * SG
                    for j in range(SG):
                        if nmm[j] == 0:
                            nc.vector.memset(aggs[j][:], 0.0)
                            transform_block(s_ * SG + j, aggs[j])
                    for k in range(NCHUNK):
                        for j in range(SG):
                            b = s_ * SG + j
                            r = int(R[b, k])
                            if r == 0:
                                continue
                            t0 = (int(off_bk[b, k]) - int(reg_off[s_, k])) // P
                            for t in range(t0, t0 + r // P):
                                done[j] += 1
                                nc.tensor.matmul(
                                    out=aggs[j][:],
                                    lhsT=g_regs[k][:, t * P : (t + 1) * P],
                                    rhs=s_regs[k][:, t * P : (t + 1) * P],
                                    start=(done[j] == 1),
                                    stop=(done[j] == nmm[j]),
                                )
                                if done[j] == nmm[j]:
                                    transform_block(b, aggs[j])

            def epi_l1i(b, y_ps):
                nc.scalar.activation(
                    out=h_buf[:, b * P : (b + 1) * P], in_=y_ps[:], func=AF.Relu,
                    scale=rini_sb[:, b : b + 1],
                )

            def epi_l1b(b, y_ps):
                rb = bp.tile([P, P], bf16, tag="rb")
                nc.scalar.activation(
                    out=rb[:], in_=y_ps[:], func=AF.Relu,
                    scale=rinb_sb[:, b : b + 1],
                )
                hs = h_buf[:, b * P : (b + 1) * P]
                nc.vector.tensor_tensor(out=hs, in0=hs, in1=rb[:], op=ALU.add)
                hn = bp.tile([P, P], bf16, tag="hn")
                nc.scalar.activation(
                    out=hn[:], in_=hs, func=AF.Copy, scale=rout_sb[:, b : b + 1]
                )
                nc.sync.dma_start(out=hn_shard[b * P : (b + 1) * P, :], in_=hn[:])

            def epi_l2(b, y_ps):
                ob = bp.tile([P, P], f32, tag="ob")
                nc.scalar.activation(
                    out=ob[:], in_=y_ps[:], func=AF.Copy,
                    scale=rini_sb[:, b : b + 1],
                )
                nc.sync.dma_start(out=y_out[b * P : (b + 1) * P, :], in_=ob[:])

            xt = [xn_i[k * chunk : (k + 1) * chunk, :] for k in range(NCHUNK)]
            xtb = [xn_b[k * chunk : (k + 1) * chunk, :] for k in range(NCHUNK)]
            ht = [hn_full[k * chunk : (k + 1) * chunk, :] for k in range(NCHUNK)]
            stage(xt, idx_i, s_i, dli_sb, lay_i, w1i_sb, b1i_sb, rvi_sb,
                  epi_l1i)
            stage(xtb, idx_b, s_b, dlb_sb, lay_b, w1b_sb, b1b_sb, rvb_sb,
                  epi_l1b)
            nc.gpsimd.collective_compute(
                "AllGather",
                mybir.AluOpType.bypass,
                replica_groups=[list(range(NCORES))],
                ins=[hn_shard[:]],
                outs=[hn_full[:]],
            )
            stage(ht, idx_i, s_i, dli_sb, lay_i, w2_sb, b2_sb, rvi_sb, epi_l2)

    nc.compile()
    return nc


def kernel(x, src_i, dst_i, src_b, dst_b, W1_i, b1_i, W1_b, b1_b, W2, b2):
    import ml_dtypes

    from concourse.bass_utils import run_bass_kernel_spmd

    bf16 = ml_dtypes.bfloat16
    x = np.asarray(x, dtype=np.float32)
    src_i = np.asarray(src_i, dtype=np.int64)
    dst_i = np.asarray(dst_i, dtype=np.int64)
    src_b = np.asarray(src_b, dtype=np.int64)
    dst_b = np.asarray(dst_b, dtype=np.int64)
    W1_i = np.asarray(W1_i, dtype=np.float32)
    b1_i = np.asarray(b1_i, dtype=np.float32)
    W1_b = np.asarray(W1_b, dtype=np.float32)
    b1_b = np.asarray(b1_b, dtype=np.float32)
    W2 = np.asarray(W2, dtype=np.float32)
    b2 = np.asarray(b2, dtype=np.float32)

    n = x.shape[0]
    npad = -(-n // (NCORES * P)) * (NCORES * P)
    shard = npad // NCORES
    nblk = shard // P
    chunk = npad // NCHUNK
    assert chunk <= 32768 and chunk % 16 == 0

    def degs(idx):
        d = np.bincount(idx, minlength=npad).astype(np.float32)
        return np.maximum(d, 1.0) ** -0.5

    ro_i = degs(src_i)
    ri_i = degs(dst_i)
    ro_b = degs(src_b)
    ri_b = degs(dst_b)

    xn_i = np.zeros((npad, P), dtype=bf16)
    xn_i[:n] = (x * ro_i[:n, None]).astype(bf16)
    xn_b = np.zeros((npad, P), dtype=bf16)
    xn_b[:n] = (x * ro_b[:n, None]).astype(bf16)

    idx_i, dl_i, lay_i = _host_prep_relation(
        src_i, dst_i, npad, shard, nblk, chunk
    )
    idx_b, dl_b, lay_b = _host_prep_relation(
        src_b, dst_b, npad, shard, nblk, chunk
    )
    s_i = idx_i.shape[2] * 16
    s_b = idx_b.shape[2] * 16

    rin_i = ri_i.reshape(NCORES, nblk, P).transpose(0, 2, 1).copy()
    rin_b = ri_b.reshape(NCORES, nblk, P).transpose(0, 2, 1).copy()
    rout2 = ro_i.reshape(NCORES, nblk, P).transpose(0, 2, 1).copy()

    def pack_rv(r):
        # block b -> partition (b % 3) * 32, cols (b // 3)*128 .. +128
        ngrp = -(-nblk // 3)
        out = np.zeros((NCORES, 65, ngrp * P), dtype=np.float32)
        rb = (1.0 / r).astype(np.float32).reshape(NCORES, nblk, P)
        for b in range(nblk):
            out[:, (b % 3) * 32, (b // 3) * P : (b // 3) * P + P] = rb[:, b, :]
        return out

    rvi_h = pack_rv(ri_i)
    rvb_h = pack_rv(ri_b)

    has_bias = bool(np.any(b1_i) or np.any(b1_b) or np.any(b2))
    key = (npad, s_i, s_b, has_bias,
           lay_i["R"].tobytes(), lay_b["R"].tobytes())
    if key not in _PROGRAM_CACHE:
        _PROGRAM_CACHE.clear()
        _PROGRAM_CACHE[key] = _build_program(
            npad, shard, nblk, chunk, s_i, s_b, lay_i, lay_b, has_bias
        )
    nc = _PROGRAM_CACHE[key]

    def bias_rep(b):
        out = np.zeros((65, P), dtype=np.float32)
        out[0] = out[32] = out[64] = b
        return out

    iota = np.tile(np.arange(P, dtype=np.float32), (P, 1)).astype(bf16)

    dl_i = dl_i.astype(bf16)
    dl_b = dl_b.astype(bf16)
    W1_i_h = W1_i.astype(bf16)
    W1_b_h = W1_b.astype(bf16)
    W2_h = W2.astype(bf16)

    in_maps = []
    for c in range(NCORES):
        in_maps.append(
            {
                "xn_i": xn_i,
                "xn_b": xn_b,
                "idx_i": idx_i[c],
                "idx_b": idx_b[c],
                "dl_i": dl_i[c],
                "dl_b": dl_b[c],
                "rin_i": rin_i[c],
                "rin_b": rin_b[c],
                "rout2": rout2[c],
                "rvi": rvi_h[c],
                "rvb": rvb_h[c],
                "w1i": W1_i_h,
                "w1b": W1_b_h,
                "w2": W2_h,
                "b1i": bias_rep(b1_i),
                "b1b": bias_rep(b1_b),
                "b2": bias_rep(b2),
                "iota": iota,
            }
        )

    import os

    trace = os.environ.get("GCN_TRACE", "0") == "1"
    res = run_bass_kernel_spmd(
        nc, in_maps, core_ids=list(range(NCORES)), trace=trace
    )
    if trace and res.exec_time_ns:
        print(f"HW exec time: {res.exec_time_ns} ns")
    y = np.concatenate([res.results[c]["y"] for c in range(NCORES)], axis=0)
    return y[:n]


# revision 38
# speedup vs baseline: 1.3609x; 1.0867x over previous
"""Trainium2 Bass kernel for a 2-layer hetero GCN (nn_NetGCN).

Math (per relation r with edges (src, dst), weights W, bias b):
    y = relu?( Dk^-1/2 * segsum_dst( (Do^-1/2 * x)[src] ) @ W + b )
Layer 1: y_i + y_b (relations 'interacts' and 'behave', relu inside each).
Layer 2: relation 'interacts' on h, no relu.

Distribution: edges sharded by dst across 8 cores (each core owns a
contiguous 12544-node slice).  Each core gathers source rows (bf16) from a
replicated node-feature table with `dma_gather`, segment-sums them into PSUM
via one-hot matmuls on TensorE, applies norm/weights, and the h-table is
AllGathered between the layers.

Slots are laid out in supergroups of SG=7 dst-blocks so each (supergroup,
chunk) is ONE big dma_gather call (168/stage instead of 1176) and each
supergroup is ONE is_equal one-hot build on DVE; block boundaries inside a
gather region are handled with partition-range matmul fragments accumulating
into per-block PSUM tiles.
"""

import sys

sys.path.insert(0, "/opt/trn_rl_repo")

import numpy as np

P = 128
NCORES = 8
NCHUNK = 4
SG = 7  # dst blocks per supergroup

_PROGRAM_CACHE = {}


def _host_prep_relation(src, dst, npad, shard, nblk, chunk, chunk_map=None):
    """Sort one relation's edges by (dst-block, src-chunk); build per-core
    int16 gather indices and per-slot dst-local columns in supergroup layout.

    Slot order: for sg: for chunk k: for block b in sg: R[b,k] slots
    (R = max edge count over cores), each (sg,k) region padded to 128.

    Returns (idx16 [NCORES,128,S//16], dstloc [NCORES,128,S//128],
             layout dict with R, rows_sgk, reg_len, reg_off, off_bk)
    """
    blk = dst // P
    if chunk_map is None:
        chk = src // chunk
        loc = src - chk * chunk
    else:
        chk, loc = chunk_map
    order = np.lexsort((chk, blk))
    loc_s = loc[order]
    dst_s = dst[order]

    nblk_tot = npad // P
    grp = blk[order] * NCHUNK + chk[order]
    counts = np.bincount(grp, minlength=nblk_tot * NCHUNK).reshape(
        nblk_tot, NCHUNK
    )
    bpc = nblk
    counts_c = counts.reshape(NCORES, bpc, NCHUNK)
    R = counts_c.max(axis=0).astype(np.int64)          # [bpc, NCHUNK]
    # 128-align block regions: every matmul fragment is a full base-0 tile
    R = -(-R // 128) * 128

    nsg = bpc // SG
    assert nsg * SG == bpc
    rows_sgk = R.reshape(nsg, SG, NCHUNK).sum(axis=1)  # [nsg, NCHUNK]
    reg_len = -(-rows_sgk // P) * P                    # [nsg, NCHUNK]

    S = int(reg_len.sum())
    reg_off = np.zeros((nsg, NCHUNK), dtype=np.int64)
    pos = 0
    for s_ in range(nsg):
        for k in range(NCHUNK):
            reg_off[s_, k] = pos
            pos += int(reg_len[s_, k])
    off_bk = np.zeros((bpc, NCHUNK), dtype=np.int64)
    for s_ in range(nsg):
        for k in range(NCHUNK):
            cur = int(reg_off[s_, k])
            for j in range(SG):
                b = s_ * SG + j
                off_bk[b, k] = cur
                cur += int(R[b, k])

    grp_start = np.zeros(nblk_tot * NCHUNK + 1, dtype=np.int64)
    np.cumsum(counts.ravel(), out=grp_start[1:])
    counts_r = counts.ravel()

    idx16 = np.zeros((NCORES, S), dtype=np.int16)
    dstloc = np.full((NCORES, S), -1.0, dtype=np.float32)
    for c in range(NCORES):
        for b in range(bpc):
            gb = c * bpc + b
            for k in range(NCHUNK):
                g = gb * NCHUNK + k
                n = int(counts_r[g])
                if n:
                    e0 = int(grp_start[g])
                    s0 = int(off_bk[b, k])
                    idx16[c, s0 : s0 + n] = loc_s[e0 : e0 + n].astype(np.int16)
                    dstloc[c, s0 : s0 + n] = (
                        dst_s[e0 : e0 + n] - (c * shard + b * P)
                    ).astype(np.float32)

    # wrap: idx j -> [j % 16, j // 16], replicated to all 8 Q7 core groups
    idx_w = np.ascontiguousarray(
        np.tile(idx16.reshape(NCORES, S // 16, 16).transpose(0, 2, 1), (1, 8, 1))
    )
    # dstloc: slot j -> [j % 128, j // 128]
    dst_w = np.ascontiguousarray(
        dstloc.reshape(NCORES, S // P, P).transpose(0, 2, 1)
    )
    layout = dict(R=R, rows_sgk=rows_sgk, reg_len=reg_len,
                  reg_off=reg_off, off_bk=off_bk)
    return idx_w, dst_w, layout


def _build_program(npad, shard, nblk, chunk, s_i, s_b, lay_i, lay_b, has_bias):
    import concourse.bacc as bacc
    import concourse.tile as tile
    from concourse import library_config, mybir

    f32 = mybir.dt.float32
    bf16 = mybir.dt.bfloat16
    i16 = mybir.dt.int16
    AF = mybir.ActivationFunctionType
    ALU = mybir.AluOpType

    nsg = nblk // SG

    nc = bacc.Bacc(
        "TRN2",
        target_bir_lowering=False,
        debug=False,
        num_devices=NCORES,
        num_swdge_queues=4,
    )

    xn_i = nc.declare_dram_parameter("xn_i", [npad, P], bf16, isOutput=False)
    xn_b = nc.declare_dram_parameter("xn_b", [npad, P], bf16, isOutput=False)
    idx_i = nc.declare_dram_parameter("idx_i", [P, s_i // 16], i16, isOutput=False)
    idx_b = nc.declare_dram_parameter("idx_b", [P, s_b // 16], i16, isOutput=False)
    dl_i = nc.declare_dram_parameter("dl_i", [P, s_i // P], bf16, isOutput=False)
    dl_b = nc.declare_dram_parameter("dl_b", [P, s_b // P], bf16, isOutput=False)
    rin_i = nc.declare_dram_parameter("rin_i", [P, nblk], f32, isOutput=False)
    rin_b = nc.declare_dram_parameter("rin_b", [P, nblk], f32, isOutput=False)
    rout2 = nc.declare_dram_parameter("rout2", [P, nblk], f32, isOutput=False)
    rvrows = 65
    rvcols = -(-nblk // 3) * P
    rvi = nc.declare_dram_parameter("rvi", [rvrows, rvcols], f32, isOutput=False)
    rvb = nc.declare_dram_parameter("rvb", [rvrows, rvcols], f32, isOutput=False)
    w1i = nc.declare_dram_parameter("w1i", [P, P], bf16, isOutput=False)
    w1b = nc.declare_dram_parameter("w1b", [P, P], bf16, isOutput=False)
    w2 = nc.declare_dram_parameter("w2", [P, P], bf16, isOutput=False)
    b1i = nc.declare_dram_parameter("b1i", [rvrows, P], f32, isOutput=False)
    b1b = nc.declare_dram_parameter("b1b", [rvrows, P], f32, isOutput=False)
    b2 = nc.declare_dram_parameter("b2", [rvrows, P], f32, isOutput=False)
    iota_in = nc.declare_dram_parameter("iota", [P, P], bf16, isOutput=False)
    y_out = nc.declare_dram_parameter("y", [shard, P], f32, isOutput=True)

    hn_shard = nc.dram_tensor("hn_shard", [shard, P], bf16)
    hn_full = nc.dram_tensor("hn_full", [npad, P], bf16, addr_space="Shared")

    max_sg_len = 0
    max_icols = 0
    for lay in (lay_i, lay_b):
        for s_ in range(nsg):
            max_sg_len = max(max_sg_len, int(lay["reg_len"][s_].sum()))
            for k in range(NCHUNK):
                max_icols = max(
                    max_icols, -(-int(lay["rows_sgk"][s_, k]) // 16)
                )

    with tile.TileContext(nc) as tc:
        nc.gpsimd.load_library(library_config.mlp)
        with (
            tc.tile_pool(name="cst", bufs=1) as cst,
            tc.tile_pool(name="edg", bufs=2) as edg,
            tc.tile_pool(name="gp", bufs=6) as gp,
            tc.tile_pool(name="sp", bufs=4) as sp,
            tc.tile_pool(name="bp", bufs=4) as bp,
            tc.tile_pool(name="pa", bufs=SG, space="PSUM") as pa,
            tc.tile_pool(name="py", bufs=1, space="PSUM") as py,
        ):
            def load_cst(t, shape, dtype=f32):
                s = cst.tile(list(shape), dtype, tag=t.name)
                nc.sync.dma_start(out=s[:], in_=t[:])
                return s

            iota_sb = load_cst(iota_in, [P, P], bf16)
            w1i_sb = load_cst(w1i, [P, P], bf16)
            w1b_sb = load_cst(w1b, [P, P], bf16)
            w2_sb = load_cst(w2, [P, P], bf16)
            b1i_sb = load_cst(b1i, [rvrows, P])
            b1b_sb = load_cst(b1b, [rvrows, P])
            b2_sb = load_cst(b2, [rvrows, P])
            rini_sb = load_cst(rin_i, [P, nblk])
            rinb_sb = load_cst(rin_b, [P, nblk])
            rout_sb = load_cst(rout2, [P, nblk])
            rvi_sb = load_cst(rvi, [rvrows, rvcols]) if has_bias else None
            rvb_sb = load_cst(rvb, [rvrows, rvcols]) if has_bias else None
            dli_sb = load_cst(dl_i, [P, s_i // P], bf16)
            dlb_sb = load_cst(dl_b, [P, s_b // P], bf16)

            h_buf = cst.tile([P, nblk * P], bf16, tag="h_buf")
            qctr = [0]

            def stage(tables, idx_t, s_len, dl_sb, lay, w_sb, bias_sb, rinv_sb,
                      epilogue):
                R = lay["R"]
                rows_sgk = lay["rows_sgk"]
                reg_len = lay["reg_len"]
                reg_off = lay["reg_off"]
                off_bk = lay["off_bk"]

                # whole-stage index table resident in SBUF (one large load)
                idx_res = cst.tile(
                    [P, s_len // 16], i16, tag="idx_res", bufs=2,
                    padded_shape=[P, max(s_i, s_b) // 16],
                )
                nc.sync.dma_start(out=idx_res[:], in_=idx_t[:])

                SUB = 1 << 30  # whole-region calls; multi-packet drain

                def transform_block(b, agg):
                    aggT = bp.tile([P, P], bf16, tag="aggT")
                    nc.scalar.copy(out=aggT[:], in_=agg[:])
                    y_ps = py.tile([P, P], f32, tag="yps")
                    nc.tensor.matmul(
                        out=y_ps[:], lhsT=aggT[:], rhs=w_sb[:],
                        start=True, stop=not has_bias,
                    )
                    if has_bias:
                        nc.tensor.matmul(
                            out=y_ps[:],
                            lhsT=rinv_sb[
                                (b % 3) * 32 : (b % 3) * 32 + 1,
                                (b // 3) * P : (b // 3) * P + P,
                            ],
                            rhs=bias_sb[(b % 3) * 32 : (b % 3) * 32 + 1, :],
                            start=False, stop=True,
                        )
                    epilogue(b, y_ps)

                for s_ in range(nsg):
                    # gathers: one tile per (sg, chunk) region, split <=1024
                    g_regs = []
                    for k in range(NCHUNK):
                        rows = int(rows_sgk[s_, k])
                        rl = int(reg_len[s_, k])
                        if rows == 0:
                            g_regs.append(None)
                            continue
                        off = int(reg_off[s_, k])
                        g_k = gp.tile(
                            [P, rl], bf16, tag="g",
                            padded_shape=[P, int(reg_len.max())],
                        )
                        g_regs.append(g_k)
                        for q in range(0, rows, SUB):
                            rq = min(SUB, rows - q)
                            nc.gpsimd.dma_gather(
                                out_ap=g_k[:, q : q + rq].rearrange(
                                    "p (t d) -> p t d", d=P
                                ),
                                in_ap=tables[k],
                                idxs_ap=idx_res[
                                    :, (off + q) // 16 : (off + q + rq) // 16
                                ],
                                num_idxs=rq,
                                num_idxs_reg=rq,
                                elem_size=P,
                                single_packet=False,
                                queue_num=qctr[0] % 4,
                            )
                            qctr[0] += 1

                    # one-hot per region on DVE
                    s_regs = []
                    for k in range(NCHUNK):
                        rl = int(reg_len[s_, k])
                        if rl == 0:
                            s_regs.append(None)
                            continue
                        off = int(reg_off[s_, k])
                        tkr = rl // P
                        s_k = sp.tile(
                            [P, rl], bf16, tag="s",
                            padded_shape=[P, int(reg_len.max())],
                        )
                        s_regs.append(s_k)
                        nc.vector.tensor_tensor(
                            out=s_k[:].rearrange("p (t n) -> p t n", n=P),
                            in0=dl_sb[:, off // P : off // P + tkr]
                            .unsqueeze(2)
                            .to_broadcast([P, tkr, P]),
                            in1=iota_sb[:]
                            .unsqueeze(1)
                            .to_broadcast([P, tkr, P]),
                            op=ALU.is_equal,
                        )

                    # region-major matmuls, one PSUM bank per block; a
                    # block's accumulation group stays open across regions
                    nmm = [
                        sum(int(R[s_ * SG + j, k]) // P for k in range(NCHUNK))
                        for j in range(SG)
                    ]
                    aggs = [
                        pa.tile([P, P], f32, tag="agg", name="agg")
                        for _ in range(SG)
                    ]
                    done = [0] * SG
                    for j in range(SG):
                        if nmm[j] == 0:
                            nc.vector.memset(aggs[j][:], 0.0)
                            transform_block(s_ * SG + j, aggs[j])
                    for k in range(NCHUNK):
                        for j in range(SG):
                            b = s_ * SG + j
                            r = int(R[b, k])
                            if r == 0:
                                continue
                            t0 = (int(off_bk[b, k]) - int(reg_off[s_, k])) // P
                            for t in range(t0, t0 + r // P):
                                done[j] += 1
                                nc.tensor.matmul(
                                    out=aggs[j][:],
                                    lhsT=g_regs[k][:, t * P : (t + 1) * P],
                                    rhs=s_regs[k][:, t * P : (t + 1) * P],
                                    start=(done[j] == 1),
                                    stop=(done[j] == nmm[j]),
                                )
                                if done[j] == nmm[j]:
                                    transform_block(b, aggs[j])

            def epi_l1i(b, y_ps):
                nc.scalar.activation(
                    out=h_buf[:, b * P : (b + 1) * P], in_=y_ps[:], func=AF.Relu,
                    scale=rini_sb[:, b : b + 1],
                )

            def epi_l1b(b, y_ps):
                rb = bp.tile([P, P], bf16, tag="rb")
                nc.scalar.activation(
                    out=rb[:], in_=y_ps[:], func=AF.Relu,
                    scale=rinb_sb[:, b : b + 1],
                )
                hs = h_buf[:, b * P : (b + 1) * P]
                nc.vector.tensor_tensor(out=hs, in0=hs, in1=rb[:], op=ALU.add)
                hn = bp.tile([P, P], bf16, tag="hn")
                nc.scalar.activation(
                    out=hn[:], in_=hs, func=AF.Copy, scale=rout_sb[:, b : b + 1]
                )
                nc.sync.dma_start(out=hn_shard[b * P : (b + 1) * P, :], in_=hn[:])

            def epi_l2(b, y_ps):
                ob = bp.tile([P, P], f32, tag="ob")
                nc.scalar.activation(
                    out=ob[:], in_=y_ps[:], func=AF.Copy,
                    scale=rini_sb[:, b : b + 1],
                )
                nc.sync.dma_start(out=y_out[b * P : (b + 1) * P, :], in_=ob[:])

            xt = [xn_i[k * chunk : (k + 1) * chunk, :] for k in range(NCHUNK)]
            xtb = [xn_b[k * chunk : (k + 1) * chunk, :] for k in range(NCHUNK)]
            ht = [hn_full[k * chunk : (k + 1) * chunk, :] for k in range(NCHUNK)]
            stage(xt, idx_i, s_i, dli_sb, lay_i, w1i_sb, b1i_sb, rvi_sb,
                  epi_l1i)
            stage(xtb, idx_b, s_b, dlb_sb, lay_b, w1b_sb, b1b_sb, rvb_sb,
                  epi_l1b)
            nc.gpsimd.collective_compute(
                "AllGather",
                mybir.AluOpType.bypass,
                replica_groups=[list(range(NCORES))],
                ins=[hn_shard[:]],
                outs=[hn_full[:]],
            )
            stage(ht, idx_i, s_i, dli_sb, lay_i, w2_sb, b2_sb, rvi_sb, epi_l2)

    nc.compile()
    return nc


def kernel(x, src_i, dst_i, src_b, dst_b, W1_i, b1_i, W1_b, b1_b, W2, b2):
    import ml_dtypes

    from concourse.bass_utils import run_bass_kernel_spmd

    bf16 = ml_dtypes.bfloat16
    x = np.asarray(x, dtype=np.float32)
    src_i = np.asarray(src_i, dtype=np.int64)
    dst_i = np.asarray(dst_i, dtype=np.int64)
    src_b = np.asarray(src_b, dtype=np.int64)
    dst_b = np.asarray(dst_b, dtype=np.int64)
    W1_i = np.asarray(W1_i, dtype=np.float32)
    b1_i = np.asarray(b1_i, dtype=np.float32)
    W1_b = np.asarray(W1_b, dtype=np.float32)
    b1_b = np.asarray(b1_b, dtype=np.float32)
    W2 = np.asarray(W2, dtype=np.float32)
    b2 = np.asarray(b2, dtype=np.float32)

    n = x.shape[0]
    npad = -(-n // (NCORES * P)) * (NCORES * P)
    shard = npad // NCORES
    nblk = shard // P
    chunk = npad // NCHUNK
    assert chunk <= 32768 and chunk % 16 == 0

    def degs(idx):
        d = np.bincount(idx, minlength=npad).astype(np.float32)
        return np.maximum(d, 1.0) ** -0.5

    ro_i = degs(src_i)
    ri_i = degs(dst_i)
    ro_b = degs(src_b)
    ri_b = degs(dst_b)

    xn_i = np.zeros((npad, P), dtype=bf16)
    xn_i[:n] = (x * ro_i[:n, None]).astype(bf16)
    xn_b = np.zeros((npad, P), dtype=bf16)
    xn_b[:n] = (x * ro_b[:n, None]).astype(bf16)

    idx_i, dl_i, lay_i = _host_prep_relation(
        src_i, dst_i, npad, shard, nblk, chunk
    )
    idx_b, dl_b, lay_b = _host_prep_relation(
        src_b, dst_b, npad, shard, nblk, chunk
    )
    s_i = idx_i.shape[2] * 16
    s_b = idx_b.shape[2] * 16

    rin_i = ri_i.reshape(NCORES, nblk, P).transpose(0, 2, 1).copy()
    rin_b = ri_b.reshape(NCORES, nblk, P).transpose(0, 2, 1).copy()
    rout2 = ro_i.reshape(NCORES, nblk, P).transpose(0, 2, 1).copy()

    def pack_rv(r):
        # block b -> partition (b % 3) * 32, cols (b // 3)*128 .. +128
        ngrp = -(-nblk // 3)
        out = np.zeros((NCORES, 65, ngrp * P), dtype=np.float32)
        rb = (1.0 / r).astype(np.float32).reshape(NCORES, nblk, P)
        for b in range(nblk):
            out[:, (b % 3) * 32, (b // 3) * P : (b // 3) * P + P] = rb[:, b, :]
        return out

    rvi_h = pack_rv(ri_i)
    rvb_h = pack_rv(ri_b)

    has_bias = bool(np.any(b1_i) or np.any(b1_b) or np.any(b2))
    key = (npad, s_i, s_b, has_bias,
           lay_i["R"].tobytes(), lay_b["R"].tobytes())
    if key not in _PROGRAM_CACHE:
        _PROGRAM_CACHE.clear()
        _PROGRAM_CACHE[key] = _build_program(
            npad, shard, nblk, chunk, s_i, s_b, lay_i, lay_b, has_bias
        )
    nc = _PROGRAM_CACHE[key]

    def bias_rep(b):
        out = np.zeros((65, P), dtype=np.float32)
        out[0] = out[32] = out[64] = b
        return out

    iota = np.tile(np.arange(P, dtype=np.float32), (P, 1)).astype(bf16)

    dl_i = dl_i.astype(bf16)
    dl_b = dl_b.astype(bf16)
    W1_i_h = W1_i.astype(bf16)
    W1_b_h = W1_b.astype(bf16)
    W2_h = W2.astype(bf16)

    in_maps = []
    for c in range(NCORES):
        in_maps.append(
            {
                "xn_i": xn_i,
                "xn_b": xn_b,
                "idx_i": idx_i[c],
                "idx_b": idx_b[c],
                "dl_i": dl_i[c],
                "dl_b": dl_b[c],
                "rin_i": rin_i[c],
                "rin_b": rin_b[c],
                "rout2": rout2[c],
                "rvi": rvi_h[c],
                "rvb": rvb_h[c],
                "w1i": W1_i_h,
                "w1b": W1_b_h,
                "w2": W2_h,
                "b1i": bias_rep(b1_i),
                "b1b": bias_rep(b1_b),
                "b2": bias_rep(b2),
                "iota": iota,
            }
        )

    import os

    trace = os.environ.get("GCN_TRACE", "0") == "1"
    res = run_bass_kernel_spmd(
        nc, in_maps, core_ids=list(range(NCORES)), trace=trace
    )
    if trace and res.exec_time_ns:
        print(f"HW exec time: {res.exec_time_ns} ns")
    y = np.concatenate([res.results[c]["y"] for c in range(NCORES)], axis=0)
    return y[:n]


# revision 41
# speedup vs baseline: 1.9161x; 1.4080x over previous
"""Trainium2 Bass kernel for a 2-layer hetero GCN (nn_NetGCN).

Math (per relation r with edges (src, dst), weights W, bias b):
    y = relu?( Dk^-1/2 * segsum_dst( (Do^-1/2 * x)[src] ) @ W + b )
Layer 1: y_i + y_b (relations 'interacts' and 'behave', relu inside each).
Layer 2: relation 'interacts' on h, no relu.

Distribution: edges sharded by dst across 8 cores (each core owns a
contiguous 12544-node slice).  Each core gathers source rows (bf16) from a
replicated node-feature table with `dma_gather`, segment-sums them into PSUM
via one-hot matmuls on TensorE, applies norm/weights, and the h-table is
AllGathered between the layers in supergroup-aligned parts that overlap
layer-2 gathers.

Slot layout: supergroups of SG=7 dst-blocks; per (supergroup, chunk) region
the blocks' edge runs are packed back-to-back at exact R (max edge count
over cores) and gathered with ONE multi-packet dma_gather call.  Matmuls
use full 128-partition tiles; a tile holding a block boundary is multiplied
once per block with a separate host-built one-hot column set (`dl` holds -1
on the other block's rows), accumulating into per-block PSUM banks.
"""

import sys

sys.path.insert(0, "/opt/trn_rl_repo")

import numpy as np

P = 128
NCORES = 8
NCHUNK = 4
SG = 7   # dst blocks per supergroup
NPART = 4  # h-table AllGather parts (supergroup-aligned)

_PROGRAM_CACHE = {}


def _mm_schedule(R, s_, k, reg_rows):
    """MM list for region (sg s_, chunk k): [(t, j, c0, c1, kmax)] where
    tile t rows [c0,c1) belong to block j and kmax is the tile's valid
    row count (last tile of the region may be partial)."""
    mms = []
    cum = 0
    for j in range(SG):
        r = int(R[s_ * SG + j, k])
        if r == 0:
            continue
        s0, s1 = cum, cum + r
        cum = s1
        for t in range(s0 // P, -(-s1 // P)):
            c0 = max(s0 - t * P, 0)
            c1 = min(s1 - t * P, P)
            kmax = min(P, reg_rows - t * P)
            mms.append((t, j, c0, c1, kmax))
    return mms


def _host_prep_relation(src, dst, npad, shard, nblk, nch, chunk_map):
    """Sort one relation's edges by (dst-block, src-chunk); build per-core
    int16 gather indices, MM-ordered dst-local columns, and the layout.

    Slot order: for sg: for chunk k: blocks of sg back-to-back at R[b,k]
    (max count over cores), each (sg,k) region padded to 128 slots.
    dl is laid out per MM column-set: column m*128+p = dst-local of the
    slot at tile-row p of MM m's tile, or -1 if outside MM m's block.
    """
    chk, loc = chunk_map
    order = np.lexsort((chk, dst // P))
    loc_s = loc[order]
    dst_s = dst[order]

    nblk_tot = npad // P
    grp = (dst[order] // P) * nch + chk[order]
    counts = np.bincount(grp, minlength=nblk_tot * nch).reshape(nblk_tot, nch)
    bpc = nblk
    counts_c = counts.reshape(NCORES, bpc, nch)
    R = counts_c.max(axis=0).astype(np.int64)          # [bpc, nch]

    nsg = bpc // SG
    assert nsg * SG == bpc
    rows_sgk = R.reshape(nsg, SG, nch).sum(axis=1)     # [nsg, nch]
    reg_len = -(-rows_sgk // P) * P

    S = int(reg_len.sum())
    reg_off = np.zeros((nsg, nch), dtype=np.int64)
    off_bk = np.zeros((bpc, nch), dtype=np.int64)
    nmm_reg = np.zeros((nsg, nch), dtype=np.int64)
    mm_off = np.zeros((nsg, nch), dtype=np.int64)
    pos = 0
    mpos = 0
    for s_ in range(nsg):
        for k in range(nch):
            reg_off[s_, k] = pos
            cur = pos
            for j in range(SG):
                b = s_ * SG + j
                off_bk[b, k] = cur
                cur += int(R[b, k])
            pos += int(reg_len[s_, k])
            mm_off[s_, k] = mpos
            nmm_reg[s_, k] = len(_mm_schedule(R, s_, k,
                                              int(rows_sgk[s_, k])))
            mpos += int(nmm_reg[s_, k])
    M = int(mpos)

    grp_start = np.zeros(nblk_tot * nch + 1, dtype=np.int64)
    np.cumsum(counts.ravel(), out=grp_start[1:])
    counts_r = counts.ravel()

    idx16 = np.zeros((NCORES, S), dtype=np.int16)
    dstloc = np.full((NCORES, S), -1.0, dtype=np.float32)
    for c in range(NCORES):
        for b in range(bpc):
            gb = c * bpc + b
            for k in range(nch):
                g = gb * nch + k
                n = int(counts_r[g])
                if n:
                    e0 = int(grp_start[g])
                    s0 = int(off_bk[b, k])
                    idx16[c, s0 : s0 + n] = loc_s[e0 : e0 + n].astype(np.int16)
                    dstloc[c, s0 : s0 + n] = (
                        dst_s[e0 : e0 + n] - (c * shard + b * P)
                    ).astype(np.float32)

    # dl in MM-column order with per-block masking
    dl_mm = np.full((NCORES, P, M), -1.0, dtype=np.float32)
    for s_ in range(nsg):
        for k in range(nch):
            off = int(reg_off[s_, k])
            for mi, (t, j, c0, c1, kmax) in enumerate(
                _mm_schedule(R, s_, k, int(rows_sgk[s_, k]))
            ):
                m = int(mm_off[s_, k]) + mi
                dl_mm[:, c0:c1, m] = dstloc[
                    :, off + t * P + c0 : off + t * P + c1
                ]

    # wrap: idx j -> [j % 16, j // 16], replicated to all 8 Q7 core groups
    idx_w = np.ascontiguousarray(
        np.tile(idx16.reshape(NCORES, S // 16, 16).transpose(0, 2, 1), (1, 8, 1))
    )
    layout = dict(R=R, rows_sgk=rows_sgk, reg_len=reg_len, reg_off=reg_off,
                  nmm_reg=nmm_reg, mm_off=mm_off, nch=nch, M=M, S=S)
    return idx_w, dl_mm, layout


def _build_program(npad, shard, nblk, chunk, lay_i, lay_b, lay_2, part_blocks,
                   has_bias):
    import concourse.bacc as bacc
    import concourse.tile as tile
    from concourse import library_config, mybir

    f32 = mybir.dt.float32
    bf16 = mybir.dt.bfloat16
    i16 = mybir.dt.int16
    AF = mybir.ActivationFunctionType
    ALU = mybir.AluOpType

    nsg = nblk // SG
    s_i, s_b, s_2 = lay_i["S"], lay_b["S"], lay_2["S"]
    m_i, m_b, m_2 = lay_i["M"], lay_b["M"], lay_2["M"]
    s_max = max(s_i, s_b, s_2)

    nc = bacc.Bacc(
        "TRN2",
        target_bir_lowering=False,
        debug=False,
        num_devices=NCORES,
        num_swdge_queues=4,
    )

    xn_i = nc.declare_dram_parameter("xn_i", [npad, P], bf16, isOutput=False)
    xn_b = nc.declare_dram_parameter("xn_b", [npad, P], bf16, isOutput=False)
    idx_i = nc.declare_dram_parameter("idx_i", [P, s_i // 16], i16, isOutput=False)
    idx_b = nc.declare_dram_parameter("idx_b", [P, s_b // 16], i16, isOutput=False)
    idx_2 = nc.declare_dram_parameter("idx_2", [P, s_2 // 16], i16, isOutput=False)
    dl_i = nc.declare_dram_parameter("dl_i", [P, m_i], bf16, isOutput=False)
    dl_b = nc.declare_dram_parameter("dl_b", [P, m_b], bf16, isOutput=False)
    dl_2 = nc.declare_dram_parameter("dl_2", [P, m_2], bf16, isOutput=False)
    rin_i = nc.declare_dram_parameter("rin_i", [P, nblk], f32, isOutput=False)
    rin_b = nc.declare_dram_parameter("rin_b", [P, nblk], f32, isOutput=False)
    rout2 = nc.declare_dram_parameter("rout2", [P, nblk], f32, isOutput=False)
    rvrows = 65
    rvcols = -(-nblk // 3) * P
    rvi = nc.declare_dram_parameter("rvi", [rvrows, rvcols], f32, isOutput=False)
    rvb = nc.declare_dram_parameter("rvb", [rvrows, rvcols], f32, isOutput=False)
    w1i = nc.declare_dram_parameter("w1i", [P, P], bf16, isOutput=False)
    w1b = nc.declare_dram_parameter("w1b", [P, P], bf16, isOutput=False)
    w2 = nc.declare_dram_parameter("w2", [P, P], bf16, isOutput=False)
    b1i = nc.declare_dram_parameter("b1i", [rvrows, P], f32, isOutput=False)
    b1b = nc.declare_dram_parameter("b1b", [rvrows, P], f32, isOutput=False)
    b2 = nc.declare_dram_parameter("b2", [rvrows, P], f32, isOutput=False)
    iota_in = nc.declare_dram_parameter("iota", [P, P], bf16, isOutput=False)
    y_out = nc.declare_dram_parameter("y", [shard, P], f32, isOutput=True)

    hn_shard = nc.dram_tensor("hn_shard", [shard, P], bf16)
    hn_part = [
        nc.dram_tensor(f"hn_part{p}", [NCORES * nb * P, P], bf16,
                       addr_space="Shared")
        for p, nb in enumerate(part_blocks)
    ]

    with tile.TileContext(nc) as tc:
        nc.gpsimd.load_library(library_config.mlp)
        with (
            tc.tile_pool(name="cst", bufs=1) as cst,
            tc.tile_pool(name="gp", bufs=6) as gp,
            tc.tile_pool(name="sp", bufs=4) as sp,
            tc.tile_pool(name="bp", bufs=4) as bp,
            tc.tile_pool(name="pa", bufs=SG, space="PSUM") as pa,
            tc.tile_pool(name="py", bufs=1, space="PSUM") as py,
        ):
            def load_cst(t, shape, dtype=f32):
                s = cst.tile(list(shape), dtype, tag=t.name)
                nc.sync.dma_start(out=s[:], in_=t[:])
                return s

            iota_sb = load_cst(iota_in, [P, P], bf16)
            w1i_sb = load_cst(w1i, [P, P], bf16)
            w1b_sb = load_cst(w1b, [P, P], bf16)
            w2_sb = load_cst(w2, [P, P], bf16)
            b1i_sb = load_cst(b1i, [rvrows, P])
            b1b_sb = load_cst(b1b, [rvrows, P])
            b2_sb = load_cst(b2, [rvrows, P])
            rini_sb = load_cst(rin_i, [P, nblk])
            rinb_sb = load_cst(rin_b, [P, nblk])
            rout_sb = load_cst(rout2, [P, nblk])
            rvi_sb = load_cst(rvi, [rvrows, rvcols]) if has_bias else None
            rvb_sb = load_cst(rvb, [rvrows, rvcols]) if has_bias else None

            h_buf = cst.tile([P, nblk * P], bf16, tag="h_buf")
            qctr = [0]

            def stage(tables, idx_t, dl_t, lay, w_sb, bias_sb, rinv_sb,
                      epilogue, after_sg=None):
                R = lay["R"]
                nch = lay["nch"]
                rows_sgk = lay["rows_sgk"]
                reg_len = lay["reg_len"]
                reg_off = lay["reg_off"]
                mm_off = lay["mm_off"]

                idx_res = cst.tile(
                    [P, lay["S"] // 16], i16, tag="idx_res", bufs=2,
                    padded_shape=[P, s_max // 16],
                )
                nc.sync.dma_start(out=idx_res[:], in_=idx_t[:])
                dl_res = cst.tile(
                    [P, lay["M"]], bf16, tag="dl_res", bufs=2,
                    padded_shape=[P, max(m_i, m_b, m_2)],
                )
                nc.sync.dma_start(out=dl_res[:], in_=dl_t[:])

                def transform_block(b, agg):
                    aggT = bp.tile([P, P], bf16, tag="aggT")
                    nc.scalar.copy(out=aggT[:], in_=agg[:])
                    y_ps = py.tile([P, P], f32, tag="yps")
                    nc.tensor.matmul(
                        out=y_ps[:], lhsT=aggT[:], rhs=w_sb[:],
                        start=True, stop=not has_bias,
                    )
                    if has_bias:
                        nc.tensor.matmul(
                            out=y_ps[:],
                            lhsT=rinv_sb[
                                (b % 3) * 32 : (b % 3) * 32 + 1,
                                (b // 3) * P : (b // 3) * P + P,
                            ],
                            rhs=bias_sb[(b % 3) * 32 : (b % 3) * 32 + 1, :],
                            start=False, stop=True,
                        )
                    epilogue(b, y_ps)

                max_rl = int(reg_len.max())
                max_nm = int(lay["nmm_reg"].max())
                for s_ in range(nsg):
                    g_regs = []
                    for k in range(nch):
                        rows = int(rows_sgk[s_, k])
                        if rows == 0:
                            g_regs.append(None)
                            continue
                        rl = int(reg_len[s_, k])
                        off = int(reg_off[s_, k])
                        g_k = gp.tile(
                            [P, rl], bf16, tag="g", padded_shape=[P, max_rl],
                        )
                        g_regs.append(g_k)
                        nc.gpsimd.dma_gather(
                            out_ap=g_k[:, :rl].rearrange(
                                "p (t d) -> p t d", d=P
                            ),
                            in_ap=tables[k],
                            idxs_ap=idx_res[
                                :, off // 16 : off // 16 + (-(-rows // 16))
                            ],
                            num_idxs=rows,
                            num_idxs_reg=rows,
                            elem_size=P,
                            single_packet=False,
                            queue_num=qctr[0] % 4,
                        )
                        qctr[0] += 1

                    s_regs = []
                    scheds = []
                    for k in range(nch):
                        rows = int(rows_sgk[s_, k])
                        sched = _mm_schedule(R, s_, k, rows)
                        scheds.append(sched)
                        nm = len(sched)
                        if nm == 0:
                            s_regs.append(None)
                            continue
                        m0 = int(mm_off[s_, k])
                        s_k = sp.tile(
                            [P, nm * P], bf16, tag="s",
                            padded_shape=[P, max_nm * P],
                        )
                        s_regs.append(s_k)
                        nc.vector.tensor_tensor(
                            out=s_k[:].rearrange("p (t n) -> p t n", n=P),
                            in0=dl_res[:, m0 : m0 + nm]
                            .unsqueeze(2)
                            .to_broadcast([P, nm, P]),
                            in1=iota_sb[:]
                            .unsqueeze(1)
                            .to_broadcast([P, nm, P]),
                            op=ALU.is_equal,
                        )

                    nmm = [0] * SG
                    for k in range(nch):
                        for (t, j, c0, c1, kmax) in scheds[k]:
                            nmm[j] += 1
                    aggs = [
                        pa.tile([P, P], f32, tag="agg", name="agg")
                        for _ in range(SG)
                    ]
                    done = [0] * SG
                    for j in range(SG):
                        if nmm[j] == 0:
                            nc.vector.memset(aggs[j][:], 0.0)
                            transform_block(s_ * SG + j, aggs[j])
                    for k in range(nch):
                        for mi, (t, j, c0, c1, kmax) in enumerate(scheds[k]):
                            done[j] += 1
                            nc.tensor.matmul(
                                out=aggs[j][:],
                                lhsT=g_regs[k][:kmax, t * P : (t + 1) * P],
                                rhs=s_regs[k][:kmax, mi * P : (mi + 1) * P],
                                start=(done[j] == 1),
                                stop=(done[j] == nmm[j]),
                            )
                            if done[j] == nmm[j]:
                                transform_block(s_ * SG + j, aggs[j])
                    if after_sg is not None:
                        after_sg(s_)

            def epi_l1i(b, y_ps):
                nc.scalar.activation(
                    out=h_buf[:, b * P : (b + 1) * P], in_=y_ps[:], func=AF.Relu,
                    scale=rini_sb[:, b : b + 1],
                )

            def epi_l1b(b, y_ps):
                rb = bp.tile([P, P], bf16, tag="rb")
                nc.scalar.activation(
                    out=rb[:], in_=y_ps[:], func=AF.Relu,
                    scale=rinb_sb[:, b : b + 1],
                )
                hs = h_buf[:, b * P : (b + 1) * P]
                nc.vector.tensor_tensor(out=hs, in0=hs, in1=rb[:], op=ALU.add)
                hn = bp.tile([P, P], bf16, tag="hn")
                nc.scalar.activation(
                    out=hn[:], in_=hs, func=AF.Copy, scale=rout_sb[:, b : b + 1]
                )
                nc.sync.dma_start(out=hn_shard[b * P : (b + 1) * P, :], in_=hn[:])

            def epi_l2(b, y_ps):
                ob = bp.tile([P, P], f32, tag="ob")
                nc.scalar.activation(
                    out=ob[:], in_=y_ps[:], func=AF.Copy,
                    scale=rini_sb[:, b : b + 1],
                )
                nc.sync.dma_start(out=y_out[b * P : (b + 1) * P, :], in_=ob[:])

            # part p covers sgs [sg0, sg1) -> blocks [sg0*SG, sg1*SG)
            part_sg_end = []
            acc = 0
            for nb in part_blocks:
                acc += nb // SG
                part_sg_end.append(acc)

            def ag_after_sg(s_):
                if s_ + 1 in part_sg_end:
                    p = part_sg_end.index(s_ + 1)
                    b0 = (part_sg_end[p - 1] * SG if p else 0)
                    b1 = part_sg_end[p] * SG
                    nc.gpsimd.collective_compute(
                        "AllGather",
                        mybir.AluOpType.bypass,
                        replica_groups=[list(range(NCORES))],
                        ins=[hn_shard[b0 * P : b1 * P, :]],
                        outs=[hn_part[p][:]],
                    )

            xt = [xn_i[k * chunk : (k + 1) * chunk, :] for k in range(NCHUNK)]
            xtb = [xn_b[k * chunk : (k + 1) * chunk, :] for k in range(NCHUNK)]
            ht = [t[:] for t in hn_part]
            stage(xt, idx_i, dl_i, lay_i, w1i_sb, b1i_sb, rvi_sb, epi_l1i)
            stage(xtb, idx_b, dl_b, lay_b, w1b_sb, b1b_sb, rvb_sb, epi_l1b,
                  after_sg=ag_after_sg)
            stage(ht, idx_2, dl_2, lay_2, w2_sb, b2_sb, rvi_sb, epi_l2)

    nc.compile()
    return nc


def kernel(x, src_i, dst_i, src_b, dst_b, W1_i, b1_i, W1_b, b1_b, W2, b2):
    import ml_dtypes

    from concourse.bass_utils import run_bass_kernel_spmd

    bf16 = ml_dtypes.bfloat16
    x = np.asarray(x, dtype=np.float32)
    src_i = np.asarray(src_i, dtype=np.int64)
    dst_i = np.asarray(dst_i, dtype=np.int64)
    src_b = np.asarray(src_b, dtype=np.int64)
    dst_b = np.asarray(dst_b, dtype=np.int64)
    W1_i = np.asarray(W1_i, dtype=np.float32)
    b1_i = np.asarray(b1_i, dtype=np.float32)
    W1_b = np.asarray(W1_b, dtype=np.float32)
    b1_b = np.asarray(b1_b, dtype=np.float32)
    W2 = np.asarray(W2, dtype=np.float32)
    b2 = np.asarray(b2, dtype=np.float32)

    n = x.shape[0]
    npad = -(-n // (NCORES * P)) * (NCORES * P)
    shard = npad // NCORES
    nblk = shard // P
    chunk = npad // NCHUNK
    assert chunk <= 32768 and chunk % 16 == 0

    def degs(idx):
        d = np.bincount(idx, minlength=npad).astype(np.float32)
        return np.maximum(d, 1.0) ** -0.5

    ro_i = degs(src_i)
    ri_i = degs(dst_i)
    ro_b = degs(src_b)
    ri_b = degs(dst_b)

    xn_i = np.zeros((npad, P), dtype=bf16)
    xn_i[:n] = (x * ro_i[:n, None]).astype(bf16)
    xn_b = np.zeros((npad, P), dtype=bf16)
    xn_b[:n] = (x * ro_b[:n, None]).astype(bf16)

    # h-table parts: supergroup-aligned split of each shard
    nsg = nblk // SG
    part_sgs = [-(-nsg // NPART)] * NPART
    part_sgs[-1] = nsg - sum(part_sgs[:-1])
    while part_sgs[-1] <= 0:  # tiny nsg fallback
        part_sgs = part_sgs[:-1]
        part_sgs[-1] = nsg - sum(part_sgs[:-1])
    part_blocks = [ps * SG for ps in part_sgs]
    npart = len(part_blocks)
    part_b0 = np.cumsum([0] + part_blocks)

    cm_i = (src_i // chunk, src_i - (src_i // chunk) * chunk)
    cm_b = (src_b // chunk, src_b - (src_b // chunk) * chunk)
    # layer-2 chunk map: node s -> (part of its block, core*qlen + local)
    blk_of = (src_i % shard) // P
    part_of = np.searchsorted(part_b0[1:], blk_of, side="right")
    core_of = src_i // shard
    qlen = np.array(part_blocks) * P
    loc2 = core_of * qlen[part_of] + (src_i % shard) - part_b0[part_of] * P
    cm_2 = (part_of, loc2)

    idx_i, dl_i_h, lay_i = _host_prep_relation(
        src_i, dst_i, npad, shard, nblk, NCHUNK, cm_i
    )
    idx_b, dl_b_h, lay_b = _host_prep_relation(
        src_b, dst_b, npad, shard, nblk, NCHUNK, cm_b
    )
    idx_2, dl_2_h, lay_2 = _host_prep_relation(
        src_i, dst_i, npad, shard, nblk, npart, cm_2
    )

    rin_i = ri_i.reshape(NCORES, nblk, P).transpose(0, 2, 1).copy()
    rin_b = ri_b.reshape(NCORES, nblk, P).transpose(0, 2, 1).copy()
    rout2 = ro_i.reshape(NCORES, nblk, P).transpose(0, 2, 1).copy()

    def pack_rv(r):
        ngrp = -(-nblk // 3)
        out = np.zeros((NCORES, 65, ngrp * P), dtype=np.float32)
        rb = (1.0 / r).astype(np.float32).reshape(NCORES, nblk, P)
        for b in range(nblk):
            out[:, (b % 3) * 32, (b // 3) * P : (b // 3) * P + P] = rb[:, b, :]
        return out

    rvi_h = pack_rv(ri_i)
    rvb_h = pack_rv(ri_b)

    has_bias = bool(np.any(b1_i) or np.any(b1_b) or np.any(b2))
    key = (npad, has_bias, tuple(part_blocks),
           lay_i["R"].tobytes(), lay_b["R"].tobytes(), lay_2["R"].tobytes())
    if key not in _PROGRAM_CACHE:
        _PROGRAM_CACHE.clear()
        _PROGRAM_CACHE[key] = _build_program(
            npad, shard, nblk, chunk, lay_i, lay_b, lay_2, part_blocks,
            has_bias
        )
    nc = _PROGRAM_CACHE[key]

    def bias_rep(b):
        out = np.zeros((65, P), dtype=np.float32)
        out[0] = out[32] = out[64] = b
        return out

    iota = np.tile(np.arange(P, dtype=np.float32), (P, 1)).astype(bf16)

    in_maps = []
    for c in range(NCORES):
        in_maps.append(
            {
                "xn_i": xn_i,
                "xn_b": xn_b,
                "idx_i": idx_i[c],
                "idx_b": idx_b[c],
                "idx_2": idx_2[c],
                "dl_i": dl_i_h[c].astype(bf16),
                "dl_b": dl_b_h[c].astype(bf16),
                "dl_2": dl_2_h[c].astype(bf16),
                "rin_i": rin_i[c],
                "rin_b": rin_b[c],
                "rout2": rout2[c],
                "rvi": rvi_h[c],
                "rvb": rvb_h[c],
                "w1i": W1_i.astype(bf16),
                "w1b": W1_b.astype(bf16),
                "w2": W2.astype(bf16),
                "b1i": bias_rep(b1_i),
                "b1b": bias_rep(b1_b),
                "b2": bias_rep(b2),
                "iota": iota,
            }
        )

    import os

    trace = os.environ.get("GCN_TRACE", "0") == "1"
    res = run_bass_kernel_spmd(
        nc, in_maps, core_ids=list(range(NCORES)), trace=trace
    )
    if trace and res.exec_time_ns:
        print(f"HW exec time: {res.exec_time_ns} ns")
    y = np.concatenate([res.results[c]["y"] for c in range(NCORES)], axis=0)
    return y[:n]
